# revision 5
# baseline (speedup 1.0000x reference)
"""Trainium2 Bass kernel: distance-decay double-softmax attention.

Reference computation per (b, c) pair (L=256, D=512):
    qkv  = x @ w_qkv;  q,k,v = split(qkv)
    attn = softmax(q @ k.T * D_h^-0.5)
    h    = relu((attn + pos) @ w1 + b1);  w = h @ w2 + b2
    attn2= softmax(attn * exp(-dist / (2 w^2 + 1e-6)))
    out  = (attn2 @ v) @ w_out + b_out

Host-side algebraic folds (exact):
    dots = q k^T * s = x (s Wq Wk^T) x^T         -> M = s*Wq@Wk.T
    y    = attn2 @ (v w_out) + b_out             -> Wv' = Wv@w_out
    (attn+pos) @ w1 + b1 = attn@w1 + (pos@w1+b1) -> P1[c] = pos[c]@w1+b1

Sharding: pure data parallel over the 128 (b,c) pairs -> 16 pairs/core,
packed as 8 "superpairs" (2 batch items of one channel share the free
dim, giving N=512 matmuls).  x arrives host-pretransposed; the output
leaves as y^T and is untransposed on the host.  attn / attn2 are
transposed on the PE (via identity).  All matmuls run as float32r
(full-rate fp32 storage) with fp32 PSUM accumulation.

Emission is software-pipelined across superpairs (stage A of superpair
sp is emitted before stage B of superpair sp-1) so the TensorEngine
never drains during the softmax/MLP chain and the HAM clock stays warm.
"""

import sys
import numpy as np

sys.path.insert(0, "/opt/trn_rl_repo")

import concourse.bass as bass  # noqa: E402,F401
import concourse.mybir as mybir  # noqa: E402
from concourse import bacc  # noqa: E402
from concourse.tile import TileContext  # noqa: E402

F32 = mybir.dt.float32
F32R = mybir.dt.float32r
AF = mybir.ActivationFunctionType
ALU = mybir.AluOpType

B, C, L, D = 8, 16, 256, 512
NCORES = 8
CH_PER_CORE = C // NCORES          # 2
NSP = (B // 2) * CH_PER_CORE       # 8 superpairs per core
P = 128
FP = 2 * L                         # 512: two pairs packed along free dim
DT = D // P                        # 4
LT = L // P                        # 2
SCALE = float(64 ** -0.5)          # DIM_HEAD ** -0.5


class _Ctx:
    pass


def _emit_stage_a(g, sp):
    """x load, t^T = (x M)^T, v' = x Wv', dots = t x^T, E=exp(dots)+rowsum."""
    nc, pp, sp_pool = g.nc, g.pp, g.apool
    MM = nc.tensor.matmul
    st = g.state[sp] = _Ctx()

    # x^T tiles [128(d), 512(l packed)]
    xt = []
    for dt in range(DT):
        t = sp_pool.tile([P, FP], F32R, tag=f"xt{dt}", name=f"xt{sp}_{dt}")
        nc.sync.dma_start(out=t[:, :], in_=g.h["x_t"][sp, dt * P:(dt + 1) * P, :])
        xt.append(t)
    st.xt = xt

    # t^T[e, l] = sum_d M[d, e] x^T[d, l]
    tT = []
    for et in range(DT):
        ps = pp.tile([P, FP], F32, tag="ps", name=f"ps_t{sp}_{et}")
        for dt in range(DT):
            MM(ps[:, :], g.m_sb[dt][:, et * P:(et + 1) * P], xt[dt][:, :],
               start=(dt == 0), stop=(dt == DT - 1))
        t = sp_pool.tile([P, FP], F32R, tag=f"tT{et}", name=f"tT{sp}_{et}")
        nc.vector.tensor_copy(t[:, :], ps[:, :])
        tT.append(t)
    st.tT = tT

    # v'[l, e] = sum_d x^T[d, l] Wv'[d, e]   (natural layout, per pair)
    v_sb = [[None] * LT for _ in range(2)]
    for pi in range(2):
        for lt in range(LT):
            ps = pp.tile([P, D], F32, tag="ps", name=f"ps_v{sp}_{pi}{lt}")
            for dt in range(DT):
                MM(ps[:, :],
                   xt[dt][:, pi * L + lt * P: pi * L + (lt + 1) * P],
                   g.wv_sb[dt][:, :],
                   start=(dt == 0), stop=(dt == DT - 1))
            t = sp_pool.tile([P, D], F32R, tag=f"v{pi}{lt}", name=f"v{sp}_{pi}{lt}")
            nc.vector.tensor_copy(t[:, :], ps[:, :])
            v_sb[pi][lt] = t
    st.v = v_sb

    # dots[i, m] = sum_e t^T[e, i] x^T[e, m]   (scale folded into M)
    dps = []
    for it in range(LT):
        ps = pp.tile([P, FP], F32, tag="ps", name=f"ps_d{sp}_{it}")
        for pi in range(2):
            o = ps[:, pi * L:(pi + 1) * L]
            for et in range(DT):
                MM(o,
                   tT[et][:, pi * L + it * P: pi * L + (it + 1) * P],
                   xt[et][:, pi * L:(pi + 1) * L],
                   start=(et == 0), stop=(et == DT - 1))
        dps.append(ps)

    # E = exp(dots), s1 = rowsum(E)
    s14 = sp_pool.tile([P, 4], F32, tag="s14", name=f"s14_{sp}")
    E = []
    for it in range(LT):
        e_t = sp_pool.tile([P, FP], F32, tag=f"E{it}", name=f"E{sp}_{it}")
        for pi in range(2):
            c = it * 2 + pi
            sl = slice(pi * L, (pi + 1) * L)
            nc.scalar.activation(e_t[:, sl], dps[it][:, sl], AF.Exp,
                                 accum_out=s14[:, c:c + 1])
        E.append(e_t)
    st.E = E
    r14 = sp_pool.tile([P, 4], F32, tag="r14", name=f"r14_{sp}")
    nc.vector.reciprocal(r14[:, :], s14[:, :])
    st.r14 = r14


def _emit_stage_b(g, sp):
    """attn, transpose, MLP, dist-decay, softmax2, transpose, y^T, DMA out."""
    nc, pp, sp_pool = g.nc, g.pp, g.sp_pool
    MM = nc.tensor.matmul
    st = g.state[sp]
    ci = sp // (NSP // CH_PER_CORE)
    E, r14 = st.E, st.r14

    # attn = E * r1  (kept for second softmax; also the transpose source)
    attn = []
    for it in range(LT):
        t = sp_pool.tile([P, FP], F32, tag=f"at{it}", name=f"attn{sp}_{it}")
        for pi in range(2):
            c = it * 2 + pi
            sl = slice(pi * L, (pi + 1) * L)
            nc.vector.tensor_scalar_mul(t[:, sl], E[it][:, sl], r14[:, c:c + 1])
        attn.append(t)
    st.attn = attn

    # attn^T  [m(part), i(packed free)]
    aT = []
    for mt in range(LT):
        ps = pp.tile([P, FP], F32, tag="ps", name=f"ps_tA{sp}_{mt}")
        for pi in range(2):
            for it in range(LT):
                nc.tensor.transpose(
                    ps[:, pi * L + it * P: pi * L + (it + 1) * P],
                    attn[it][:, pi * L + mt * P: pi * L + (mt + 1) * P],
                    g.id_sb[:, :])
        t = sp_pool.tile([P, FP], F32R, tag=f"trT{mt}", name=f"aT{sp}_{mt}")
        nc.vector.tensor_copy(t[:, :], ps[:, :])
        aT.append(t)

    # h^T = relu(w1^T attn^T + P1^T)   [j(part), i(packed)]
    hT = []
    for jt in range(LT):
        ps = pp.tile([P, FP], F32, tag="ps", name=f"ps_h{sp}_{jt}")
        for mt in range(LT):
            MM(ps[:, :], g.w1_sb[mt][:, jt * P:(jt + 1) * P], aT[mt][:, :],
               start=(mt == 0), stop=False)
        MM(ps[:, :], g.idr_sb[:, :], g.p1_sb[ci][jt][:, :],
           start=False, stop=True)
        t = sp_pool.tile([P, FP], F32R, tag=f"hT{jt}", name=f"hT{sp}_{jt}")
        nc.scalar.activation(t[:, :], ps[:, :], AF.Relu)
        hT.append(t)

    # w[i] = h[i, :] @ w2 ; negt = -1/(2(w+b2)^2 + 1e-6)
    wps = pp.tile([P, 8], F32, tag="ps", name=f"ps_w{sp}")
    for pi in range(2):
        for it in range(LT):
            c = it * 2 + pi
            for jt in range(LT):
                MM(wps[:, 2 * c:2 * c + 2],
                   hT[jt][:, pi * L + it * P: pi * L + (it + 1) * P],
                   g.w2_sb[jt][:, :],
                   start=(jt == 0), stop=(jt == LT - 1))
    w4 = sp_pool.tile([P, 8], F32, tag="w4", name=f"w4_{sp}")
    nc.scalar.activation(w4[:, :], wps[:, :], AF.Square, bias=g.b2_sb[:, 0:1])
    nc.vector.tensor_scalar(w4[:, :], w4[:, :], -2.0, -1e-6, ALU.mult, ALU.add)
    negt = sp_pool.tile([P, 8], F32, tag="negt", name=f"negt_{sp}")
    nc.vector.reciprocal(negt[:, :], w4[:, :])

    # wg = exp(dist * negt); p2 = attn*wg; E2 = exp(p2) (+s2); attn2 = E2*r2
    s24 = sp_pool.tile([P, 4], F32, tag="s24", name=f"s24_{sp}")
    wg = []
    for it in range(LT):
        t = sp_pool.tile([P, FP], F32, tag=f"wg{it}", name=f"wg{sp}_{it}")
        for pi in range(2):
            c = it * 2 + pi
            sl = slice(pi * L, (pi + 1) * L)
            nc.scalar.activation(t[:, sl], g.dist_sb[it][:, sl], AF.Exp,
                                 scale=negt[:, 2 * c:2 * c + 1])
        nc.vector.tensor_mul(t[:, :], st.attn[it][:, :], t[:, :])
        for pi in range(2):
            c = it * 2 + pi
            sl = slice(pi * L, (pi + 1) * L)
            nc.scalar.activation(t[:, sl], t[:, sl], AF.Exp,
                                 accum_out=s24[:, c:c + 1])
        wg.append(t)
    r24 = sp_pool.tile([P, 4], F32, tag="r24", name=f"r24_{sp}")
    nc.vector.reciprocal(r24[:, :], s24[:, :])
    for it in range(LT):
        for pi in range(2):
            c = it * 2 + pi
            sl = slice(pi * L, (pi + 1) * L)
            nc.vector.tensor_scalar_mul(wg[it][:, sl], wg[it][:, sl],
                                        r24[:, c:c + 1])

    # attn2^T [m(part), i(packed)]
    a2T = []
    for mt in range(LT):
        ps = pp.tile([P, FP], F32, tag="ps", name=f"ps_tB{sp}_{mt}")
        for pi in range(2):
            for it in range(LT):
                nc.tensor.transpose(
                    ps[:, pi * L + it * P: pi * L + (it + 1) * P],
                    wg[it][:, pi * L + mt * P: pi * L + (mt + 1) * P],
                    g.id_sb[:, :])
        t = sp_pool.tile([P, FP], F32R, tag=f"trT{mt}", name=f"a2T{sp}_{mt}")
        nc.vector.tensor_copy(t[:, :], ps[:, :])
        a2T.append(t)

    # y^T[d', i] = sum_m v'[m, d'] attn2^T[m, i] + b_out[d']
    for ot in range(DT):
        ps = pp.tile([P, FP], F32, tag="ps", name=f"ps_y{sp}_{ot}")
        for pi in range(2):
            o = ps[:, pi * L:(pi + 1) * L]
            for mt in range(LT):
                MM(o,
                   st.v[pi][mt][:, ot * P:(ot + 1) * P],
                   a2T[mt][:, pi * L:(pi + 1) * L],
                   start=(mt == 0), stop=(mt == LT - 1))
        yt = g.ypool.tile([P, FP], F32, tag=f"yT{ot}", name=f"yT{sp}_{ot}")
        nc.scalar.activation(yt[:, :], ps[:, :], AF.Identity,
                             bias=g.bo_sb[ot][:, 0:1])
        nc.sync.dma_start(out=g.h["out"][sp, ot * P:(ot + 1) * P, :],
                          in_=yt[:, :])


def _emit(nc, tc, h):
    import contextlib
    g = _Ctx()
    g.nc, g.h = nc, h
    g.state = {}

    with contextlib.ExitStack() as ex:
        cpool = ex.enter_context(tc.tile_pool(name="consts", bufs=1))
        g.apool = ex.enter_context(tc.tile_pool(name="astream", bufs=3))
        g.sp_pool = ex.enter_context(tc.tile_pool(name="stream", bufs=2))
        g.ypool = ex.enter_context(tc.tile_pool(name="yout", bufs=1))
        g.pp = ex.enter_context(tc.tile_pool(name="ps", bufs=8, space="PSUM"))

        # ---- constants ----
        def cload(name, shape, dt_, src):
            t = cpool.tile(shape, dt_, tag=name, name=name)
            nc.sync.dma_start(out=t[:shape[0], :], in_=src)
            return t

        g.m_sb = [cload(f"m{dt}", [P, D], F32R, h["m"][dt * P:(dt + 1) * P, :])
                  for dt in range(DT)]
        g.wv_sb = [cload(f"wv{dt}", [P, D], F32R, h["wv"][dt * P:(dt + 1) * P, :])
                   for dt in range(DT)]
        g.w1_sb = [cload(f"w1_{mt}", [P, L], F32R, h["w1"][mt * P:(mt + 1) * P, :])
                   for mt in range(LT)]
        g.w2_sb = [cload(f"w2_{jt}", [P, 2], F32R, h["w2d"][jt * P:(jt + 1) * P, :])
                   for jt in range(LT)]
        g.p1_sb = [[cload(f"p1_{ci}_{jt}", [P, FP], F32R,
                          h["p1t"][ci, jt * P:(jt + 1) * P, :])
                    for jt in range(LT)] for ci in range(CH_PER_CORE)]
        g.bo_sb = [cload(f"bout{ot}", [P, 1], F32, h["bout"][ot * P:(ot + 1) * P, :])
                   for ot in range(DT)]
        g.b2_sb = cload("b2r", [P, 1], F32, h["b2r"][:, :])
        g.id_sb = cload("ident", [P, P], F32, h["ident"][:, :])
        g.idr_sb = cload("identr", [P, P], F32R, h["identr"][:, :])
        g.dist_sb = [cload(f"dist{it}", [P, FP], F32,
                           h["dist"][it * P:(it + 1) * P, :])
                     for it in range(LT)]

        # ---- software-pipelined superpair loop (depth-2 lookahead) ----
        _emit_stage_a(g, 0)
        _emit_stage_a(g, 1)
        for sp in range(NSP):
            _emit_stage_b(g, sp)
            if sp + 2 < NSP:
                _emit_stage_a(g, sp + 2)


def build_nc():
    nc = bacc.Bacc("TRN2", target_bir_lowering=False, debug=False,
                   enable_asserts=False)
    h = {}
    h["x_t"] = nc.declare_dram_parameter("x_t", [NSP, D, FP], F32R, False)
    h["m"] = nc.declare_dram_parameter("m", [D, D], F32R, False)
    h["wv"] = nc.declare_dram_parameter("wv", [D, D], F32R, False)
    h["w1"] = nc.declare_dram_parameter("w1", [L, L], F32R, False)
    h["w2d"] = nc.declare_dram_parameter("w2d", [L, 2], F32R, False)
    h["p1t"] = nc.declare_dram_parameter("p1t", [CH_PER_CORE, L, FP], F32R, False)
    h["dist"] = nc.declare_dram_parameter("dist", [L, FP], F32, False)
    h["bout"] = nc.declare_dram_parameter("bout", [D, 1], F32, False)
    h["b2r"] = nc.declare_dram_parameter("b2r", [P, 1], F32, False)
    h["ident"] = nc.declare_dram_parameter("ident", [P, P], F32, False)
    h["identr"] = nc.declare_dram_parameter("identr", [P, P], F32R, False)
    h["out"] = nc.declare_dram_parameter("out", [NSP, D, FP], F32, True)

    with TileContext(nc) as tc:
        _emit(nc, tc, h)
    nc.compile()
    return nc


def make_in_maps(x, w_qkv, pos_emb, w1, b1, w2, b2, w_out, b_out):
    f = lambda a: np.ascontiguousarray(np.asarray(a), dtype=np.float32)
    x, w_qkv, pos_emb = f(x), f(w_qkv), f(pos_emb)
    w1, b1, w2, b2, w_out, b_out = f(w1), f(b1), f(w2), f(b2), f(w_out), f(b_out)

    wq, wk, wv = w_qkv[:, :D], w_qkv[:, D:2 * D], w_qkv[:, 2 * D:]
    m = np.ascontiguousarray((SCALE * (wq.astype(np.float64)
                                       @ wk.astype(np.float64).T))
                             .astype(np.float32))
    wvp = np.ascontiguousarray((wv.astype(np.float64)
                                @ w_out.astype(np.float64)).astype(np.float32))
    # P1[c] = pos[c] @ w1 + b1, transposed [L(j), L(i)] per channel
    p1 = pos_emb[0].astype(np.float64) @ w1.astype(np.float64) + b1
    p1t_single = np.ascontiguousarray(p1.transpose(0, 2, 1).astype(np.float32))
    idx = np.arange(L, dtype=np.float32)
    dist = (idx[None, :] - idx[:, None]) ** 2
    distp = np.ascontiguousarray(np.concatenate([dist, dist], axis=1))
    common = {
        "m": m,
        "wv": wvp,
        "w1": w1,
        "w2d": np.ascontiguousarray(np.concatenate([w2, w2], axis=1)),
        "dist": distp,
        "bout": np.ascontiguousarray(b_out.reshape(D, 1)),
        "b2r": np.full((P, 1), b2.reshape(-1)[0], np.float32),
        "ident": np.eye(P, dtype=np.float32),
        "identr": np.eye(P, dtype=np.float32),
    }
    in_maps = []
    for core in range(NCORES):
        x_t = np.empty((NSP, D, FP), np.float32)
        p1t = np.empty((CH_PER_CORE, L, FP), np.float32)
        for ci in range(CH_PER_CORE):
            ch = core * CH_PER_CORE + ci
            p1t[ci, :, :L] = p1t_single[ch]
            p1t[ci, :, L:] = p1t_single[ch]
            for bp in range(B // 2):
                s = ci * (B // 2) + bp
                x_t[s, :, :L] = x[2 * bp, ch].T
                x_t[s, :, L:] = x[2 * bp + 1, ch].T
        mcore = dict(common)
        mcore["x_t"] = x_t
        mcore["p1t"] = np.ascontiguousarray(p1t)
        in_maps.append(mcore)
    return in_maps


def assemble_out(results):
    """results: list (per core) of dicts with 'out' [NSP, D, FP]."""
    y = np.empty((B, C, L, D), np.float32)
    for core in range(NCORES):
        o = results[core]["out"]
        for ci in range(CH_PER_CORE):
            ch = core * CH_PER_CORE + ci
            for bp in range(B // 2):
                s = ci * (B // 2) + bp
                y[2 * bp, ch] = o[s, :, :L].T
                y[2 * bp + 1, ch] = o[s, :, L:].T
    return y


_NC = None
LAST_RESULT = None


def kernel(x, w_qkv, pos_emb, w1, b1, w2, b2, w_out, b_out):
    global _NC, LAST_RESULT
    from concourse.bass_utils import run_bass_kernel_spmd

    if _NC is None:
        _NC = build_nc()
    in_maps = make_in_maps(x, w_qkv, pos_emb, w1, b1, w2, b2, w_out, b_out)
    res = run_bass_kernel_spmd(_NC, in_maps, core_ids=list(range(NCORES)))
    LAST_RESULT = res
    return assemble_out(res.results)


# revision 6
# speedup vs baseline: 1.4761x; 1.4761x over previous
"""Trainium2 Bass kernel: distance-decay double-softmax attention.

Reference computation per (b, c) pair (L=256, D=512):
    qkv  = x @ w_qkv;  q,k,v = split(qkv)
    attn = softmax(q @ k.T * D_h^-0.5)
    h    = relu((attn + pos) @ w1 + b1);  w = h @ w2 + b2
    attn2= softmax(attn * exp(-dist / (2 w^2 + 1e-6)))
    out  = (attn2 @ v) @ w_out + b_out

Host-side algebraic folds (exact):
    dots = q k^T * s = x (s Wq Wk^T) x^T         -> M = s*Wq@Wk.T
    y    = attn2 @ (v w_out) + b_out             -> Wv' = Wv@w_out
    (attn+pos) @ w1 + b1 = attn@w1 + (pos@w1+b1) -> P1[c] = pos[c]@w1+b1

Sharding: pure data parallel over the 128 (b,c) pairs -> 16 pairs/core,
packed as 8 "superpairs" (2 batch items of one channel share the free
dim, giving N=512 matmuls).  x arrives host-pretransposed; the output
leaves as y^T and is untransposed on the host.  attn / attn2 are
transposed on the PE (via identity).  All matmuls run as float32r
(full-rate fp32 storage) with fp32 PSUM accumulation.

Emission is software-pipelined across superpairs (stage A of superpair
sp is emitted before stage B of superpair sp-1) so the TensorEngine
never drains during the softmax/MLP chain and the HAM clock stays warm.
"""

import sys
import numpy as np

sys.path.insert(0, "/opt/trn_rl_repo")

import concourse.bass as bass  # noqa: E402,F401
import concourse.mybir as mybir  # noqa: E402
from concourse import bacc  # noqa: E402
from concourse.tile import TileContext  # noqa: E402

F32 = mybir.dt.float32
F32R = mybir.dt.float32r
AF = mybir.ActivationFunctionType
ALU = mybir.AluOpType

B, C, L, D = 8, 16, 256, 512
NCORES = 8
CH_PER_CORE = C // NCORES          # 2
NSP = (B // 2) * CH_PER_CORE       # 8 superpairs per core
P = 128
FP = 2 * L                         # 512: two pairs packed along free dim
DT = D // P                        # 4
LT = L // P                        # 2
SCALE = float(64 ** -0.5)          # DIM_HEAD ** -0.5


class _Ctx:
    pass


def _emit_stage_a(g, sp):
    """x load, t^T = (x M)^T, v' = x Wv', dots = t x^T, E=exp(dots)+rowsum."""
    nc, pp, sp_pool = g.nc, g.pp, g.apool
    MM = nc.tensor.matmul
    st = g.state[sp] = _Ctx()

    # x^T tiles [128(d), 512(l packed)]
    xt = []
    for dt in range(DT):
        t = sp_pool.tile([P, FP], F32R, tag=f"xt{dt}", name=f"xt{sp}_{dt}")
        nc.sync.dma_start(out=t[:, :], in_=g.h["x_t"][sp, dt * P:(dt + 1) * P, :])
        xt.append(t)
    st.xt = xt

    # t^T[e, l] = sum_d M[d, e] x^T[d, l]
    tT = []
    for et in range(DT):
        ps = pp.tile([P, FP], F32, tag="ps", name=f"ps_t{sp}_{et}")
        for dt in range(DT):
            MM(ps[:, :], g.m_sb[dt][:, et * P:(et + 1) * P], xt[dt][:, :],
               start=(dt == 0), stop=(dt == DT - 1))
        t = sp_pool.tile([P, FP], F32R, tag=f"tT{et}", name=f"tT{sp}_{et}")
        nc.vector.tensor_copy(t[:, :], ps[:, :])
        tT.append(t)
    st.tT = tT

    # v'[l, e] = sum_d x^T[d, l] Wv'[d, e]   (natural layout, per pair)
    v_sb = [[None] * LT for _ in range(2)]
    for pi in range(2):
        for lt in range(LT):
            ps = pp.tile([P, D], F32, tag="ps", name=f"ps_v{sp}_{pi}{lt}")
            for dt in range(DT):
                MM(ps[:, :],
                   xt[dt][:, pi * L + lt * P: pi * L + (lt + 1) * P],
                   g.wv_sb[dt][:, :],
                   start=(dt == 0), stop=(dt == DT - 1))
            t = sp_pool.tile([P, D], F32R, tag=f"v{pi}{lt}", name=f"v{sp}_{pi}{lt}")
            nc.vector.tensor_copy(t[:, :], ps[:, :])
            v_sb[pi][lt] = t
    st.v = v_sb

    # dots[i, m] = sum_e t^T[e, i] x^T[e, m]   (scale folded into M)
    dps = []
    for it in range(LT):
        ps = pp.tile([P, FP], F32, tag="ps", name=f"ps_d{sp}_{it}")
        for pi in range(2):
            o = ps[:, pi * L:(pi + 1) * L]
            for et in range(DT):
                MM(o,
                   tT[et][:, pi * L + it * P: pi * L + (it + 1) * P],
                   xt[et][:, pi * L:(pi + 1) * L],
                   start=(et == 0), stop=(et == DT - 1))
        dps.append(ps)

    # E = exp(dots), s1 = rowsum(E)
    s14 = sp_pool.tile([P, 4], F32, tag="s14", name=f"s14_{sp}")
    E = []
    for it in range(LT):
        e_t = sp_pool.tile([P, FP], F32, tag=f"E{it}", name=f"E{sp}_{it}")
        for pi in range(2):
            c = it * 2 + pi
            sl = slice(pi * L, (pi + 1) * L)
            nc.scalar.activation(e_t[:, sl], dps[it][:, sl], AF.Exp,
                                 accum_out=s14[:, c:c + 1])
        E.append(e_t)
    st.E = E
    r14 = sp_pool.tile([P, 4], F32, tag="r14", name=f"r14_{sp}")
    nc.vector.reciprocal(r14[:, :], s14[:, :])
    st.r14 = r14

    # attn = E * r1  (used by both the MLP transpose and the second softmax)
    attn = []
    for it in range(LT):
        t = sp_pool.tile([P, FP], F32, tag=f"at{it}", name=f"attn{sp}_{it}")
        for pi in range(2):
            c = it * 2 + pi
            sl = slice(pi * L, (pi + 1) * L)
            nc.vector.tensor_scalar_mul(t[:, sl], E[it][:, sl], r14[:, c:c + 1])
        attn.append(t)
    st.attn = attn


def _emit_stage_b1(g, sp):
    """transpose attn, MLP, dist-decay, softmax2 -> attn2 (in wg tiles)."""
    nc, pp, sp_pool = g.nc, g.pp, g.sp_pool
    MM = nc.tensor.matmul
    st = g.state[sp]
    ci = sp // (NSP // CH_PER_CORE)
    attn = st.attn

    # attn^T  [m(part), i(packed free)]
    aT = []
    for mt in range(LT):
        ps = pp.tile([P, FP], F32, tag="ps", name=f"ps_tA{sp}_{mt}")
        for pi in range(2):
            for it in range(LT):
                nc.tensor.transpose(
                    ps[:, pi * L + it * P: pi * L + (it + 1) * P],
                    attn[it][:, pi * L + mt * P: pi * L + (mt + 1) * P],
                    g.id_sb[:, :])
        t = sp_pool.tile([P, FP], F32R, tag=f"trT{mt}", name=f"aT{sp}_{mt}")
        nc.vector.tensor_copy(t[:, :], ps[:, :])
        aT.append(t)

    # h^T = relu(w1^T attn^T + P1^T)   [j(part), i(packed)]
    hT = []
    for jt in range(LT):
        ps = pp.tile([P, FP], F32, tag="ps", name=f"ps_h{sp}_{jt}")
        for mt in range(LT):
            MM(ps[:, :], g.w1_sb[mt][:, jt * P:(jt + 1) * P], aT[mt][:, :],
               start=(mt == 0), stop=False)
        MM(ps[:, :], g.idr_sb[:, :], g.p1_sb[ci][jt][:, :],
           start=False, stop=True)
        t = sp_pool.tile([P, FP], F32R, tag=f"hT{jt}", name=f"hT{sp}_{jt}")
        nc.scalar.activation(t[:, :], ps[:, :], AF.Relu)
        hT.append(t)

    # w[i] = h[i, :] @ w2 ; negt = -1/(2(w+b2)^2 + 1e-6)
    wps = pp.tile([P, 8], F32, tag="ps", name=f"ps_w{sp}")
    for pi in range(2):
        for it in range(LT):
            c = it * 2 + pi
            for jt in range(LT):
                MM(wps[:, 2 * c:2 * c + 2],
                   hT[jt][:, pi * L + it * P: pi * L + (it + 1) * P],
                   g.w2_sb[jt][:, :],
                   start=(jt == 0), stop=(jt == LT - 1))
    w4 = sp_pool.tile([P, 8], F32, tag="w4", name=f"w4_{sp}")
    nc.scalar.activation(w4[:, :], wps[:, :], AF.Square, bias=g.b2_sb[:, 0:1])
    nc.vector.tensor_scalar(w4[:, :], w4[:, :], -2.0, -1e-6, ALU.mult, ALU.add)
    negt = sp_pool.tile([P, 8], F32, tag="negt", name=f"negt_{sp}")
    nc.vector.reciprocal(negt[:, :], w4[:, :])

    # wg = exp(dist * negt); p2 = attn*wg; E2 = exp(p2) (+s2); attn2 = E2*r2
    s24 = sp_pool.tile([P, 4], F32, tag="s24", name=f"s24_{sp}")
    wg = []
    for it in range(LT):
        t = sp_pool.tile([P, FP], F32, tag=f"wg{it}", name=f"wg{sp}_{it}")
        for pi in range(2):
            c = it * 2 + pi
            sl = slice(pi * L, (pi + 1) * L)
            nc.scalar.activation(t[:, sl], g.dist_sb[it][:, sl], AF.Exp,
                                 scale=negt[:, 2 * c:2 * c + 1])
        nc.vector.tensor_mul(t[:, :], st.attn[it][:, :], t[:, :])
        for pi in range(2):
            c = it * 2 + pi
            sl = slice(pi * L, (pi + 1) * L)
            nc.scalar.activation(t[:, sl], t[:, sl], AF.Exp,
                                 accum_out=s24[:, c:c + 1])
        wg.append(t)
    r24 = sp_pool.tile([P, 4], F32, tag="r24", name=f"r24_{sp}")
    nc.vector.reciprocal(r24[:, :], s24[:, :])
    for it in range(LT):
        for pi in range(2):
            c = it * 2 + pi
            sl = slice(pi * L, (pi + 1) * L)
            nc.vector.tensor_scalar_mul(wg[it][:, sl], wg[it][:, sl],
                                        r24[:, c:c + 1])

    st.wg = wg


def _emit_stage_b2(g, sp):
    """transpose attn2, y^T matmuls, bias, DMA out."""
    nc, pp, sp_pool = g.nc, g.pp, g.sp_pool
    MM = nc.tensor.matmul
    st = g.state[sp]
    wg = st.wg

    # attn2^T [m(part), i(packed)]
    a2T = []
    for mt in range(LT):
        ps = pp.tile([P, FP], F32, tag="ps", name=f"ps_tB{sp}_{mt}")
        for pi in range(2):
            for it in range(LT):
                nc.tensor.transpose(
                    ps[:, pi * L + it * P: pi * L + (it + 1) * P],
                    wg[it][:, pi * L + mt * P: pi * L + (mt + 1) * P],
                    g.id_sb[:, :])
        t = sp_pool.tile([P, FP], F32R, tag=f"trT{mt}", name=f"a2T{sp}_{mt}")
        nc.vector.tensor_copy(t[:, :], ps[:, :])
        a2T.append(t)

    # y^T[d', i] = sum_m v'[m, d'] attn2^T[m, i] + b_out[d']
    for ot in range(DT):
        ps = pp.tile([P, FP], F32, tag="ps", name=f"ps_y{sp}_{ot}")
        for pi in range(2):
            o = ps[:, pi * L:(pi + 1) * L]
            for mt in range(LT):
                MM(o,
                   st.v[pi][mt][:, ot * P:(ot + 1) * P],
                   a2T[mt][:, pi * L:(pi + 1) * L],
                   start=(mt == 0), stop=(mt == LT - 1))
        yt = g.ypool.tile([P, FP], F32, tag=f"yT{ot}", name=f"yT{sp}_{ot}")
        nc.scalar.activation(yt[:, :], ps[:, :], AF.Identity,
                             bias=g.bo_sb[ot][:, 0:1])
        nc.sync.dma_start(out=g.h["out"][sp, ot * P:(ot + 1) * P, :],
                          in_=yt[:, :])


def _emit(nc, tc, h):
    import contextlib
    g = _Ctx()
    g.nc, g.h = nc, h
    g.state = {}

    with contextlib.ExitStack() as ex:
        cpool = ex.enter_context(tc.tile_pool(name="consts", bufs=1))
        g.apool = ex.enter_context(tc.tile_pool(name="astream", bufs=3))
        g.sp_pool = ex.enter_context(tc.tile_pool(name="stream", bufs=2))
        g.ypool = ex.enter_context(tc.tile_pool(name="yout", bufs=1))
        g.pp = ex.enter_context(tc.tile_pool(name="ps", bufs=8, space="PSUM"))

        # ---- constants ----
        def cload(name, shape, dt_, src):
            t = cpool.tile(shape, dt_, tag=name, name=name)
            nc.sync.dma_start(out=t[:shape[0], :], in_=src)
            return t

        g.m_sb = [cload(f"m{dt}", [P, D], F32R, h["m"][dt * P:(dt + 1) * P, :])
                  for dt in range(DT)]
        g.wv_sb = [cload(f"wv{dt}", [P, D], F32R, h["wv"][dt * P:(dt + 1) * P, :])
                   for dt in range(DT)]
        g.w1_sb = [cload(f"w1_{mt}", [P, L], F32R, h["w1"][mt * P:(mt + 1) * P, :])
                   for mt in range(LT)]
        g.w2_sb = [cload(f"w2_{jt}", [P, 2], F32R, h["w2d"][jt * P:(jt + 1) * P, :])
                   for jt in range(LT)]
        g.p1_sb = [[cload(f"p1_{ci}_{jt}", [P, FP], F32R,
                          h["p1t"][ci, jt * P:(jt + 1) * P, :])
                    for jt in range(LT)] for ci in range(CH_PER_CORE)]
        g.bo_sb = [cload(f"bout{ot}", [P, 1], F32, h["bout"][ot * P:(ot + 1) * P, :])
                   for ot in range(DT)]
        g.b2_sb = cload("b2r", [P, 1], F32, h["b2r"][:, :])
        g.id_sb = cload("ident", [P, P], F32, h["ident"][:, :])
        g.idr_sb = cload("identr", [P, P], F32R, h["identr"][:, :])
        g.dist_sb = [cload(f"dist{it}", [P, FP], F32,
                           h["dist"][it * P:(it + 1) * P, :])
                     for it in range(LT)]

        # ---- software-pipelined superpair loop ----
        # PE stream per period: [B1(sp)] [A(sp+2): 48 independent MMs]
        # [B2(sp)] -- the A block covers the softmax/MLP chain latency so
        # B2's transposes never stall the PE.
        _emit_stage_a(g, 0)
        _emit_stage_a(g, 1)
        for sp in range(NSP):
            _emit_stage_b1(g, sp)
            if sp + 2 < NSP:
                _emit_stage_a(g, sp + 2)
            _emit_stage_b2(g, sp)


def build_nc():
    nc = bacc.Bacc("TRN2", target_bir_lowering=False, debug=False,
                   enable_asserts=False)
    h = {}
    h["x_t"] = nc.declare_dram_parameter("x_t", [NSP, D, FP], F32R, False)
    h["m"] = nc.declare_dram_parameter("m", [D, D], F32R, False)
    h["wv"] = nc.declare_dram_parameter("wv", [D, D], F32R, False)
    h["w1"] = nc.declare_dram_parameter("w1", [L, L], F32R, False)
    h["w2d"] = nc.declare_dram_parameter("w2d", [L, 2], F32R, False)
    h["p1t"] = nc.declare_dram_parameter("p1t", [CH_PER_CORE, L, FP], F32R, False)
    h["dist"] = nc.declare_dram_parameter("dist", [L, FP], F32, False)
    h["bout"] = nc.declare_dram_parameter("bout", [D, 1], F32, False)
    h["b2r"] = nc.declare_dram_parameter("b2r", [P, 1], F32, False)
    h["ident"] = nc.declare_dram_parameter("ident", [P, P], F32, False)
    h["identr"] = nc.declare_dram_parameter("identr", [P, P], F32R, False)
    h["out"] = nc.declare_dram_parameter("out", [NSP, D, FP], F32, True)

    with TileContext(nc) as tc:
        _emit(nc, tc, h)
    nc.compile()
    return nc


def make_in_maps(x, w_qkv, pos_emb, w1, b1, w2, b2, w_out, b_out):
    f = lambda a: np.ascontiguousarray(np.asarray(a), dtype=np.float32)
    x, w_qkv, pos_emb = f(x), f(w_qkv), f(pos_emb)
    w1, b1, w2, b2, w_out, b_out = f(w1), f(b1), f(w2), f(b2), f(w_out), f(b_out)

    wq, wk, wv = w_qkv[:, :D], w_qkv[:, D:2 * D], w_qkv[:, 2 * D:]
    m = np.ascontiguousarray((SCALE * (wq.astype(np.float64)
                                       @ wk.astype(np.float64).T))
                             .astype(np.float32))
    wvp = np.ascontiguousarray((wv.astype(np.float64)
                                @ w_out.astype(np.float64)).astype(np.float32))
    # P1[c] = pos[c] @ w1 + b1, transposed [L(j), L(i)] per channel
    p1 = pos_emb[0].astype(np.float64) @ w1.astype(np.float64) + b1
    p1t_single = np.ascontiguousarray(p1.transpose(0, 2, 1).astype(np.float32))
    idx = np.arange(L, dtype=np.float32)
    dist = (idx[None, :] - idx[:, None]) ** 2
    distp = np.ascontiguousarray(np.concatenate([dist, dist], axis=1))
    common = {
        "m": m,
        "wv": wvp,
        "w1": w1,
        "w2d": np.ascontiguousarray(np.concatenate([w2, w2], axis=1)),
        "dist": distp,
        "bout": np.ascontiguousarray(b_out.reshape(D, 1)),
        "b2r": np.full((P, 1), b2.reshape(-1)[0], np.float32),
        "ident": np.eye(P, dtype=np.float32),
        "identr": np.eye(P, dtype=np.float32),
    }
    in_maps = []
    for core in range(NCORES):
        x_t = np.empty((NSP, D, FP), np.float32)
        p1t = np.empty((CH_PER_CORE, L, FP), np.float32)
        for ci in range(CH_PER_CORE):
            ch = core * CH_PER_CORE + ci
            p1t[ci, :, :L] = p1t_single[ch]
            p1t[ci, :, L:] = p1t_single[ch]
            for bp in range(B // 2):
                s = ci * (B // 2) + bp
                x_t[s, :, :L] = x[2 * bp, ch].T
                x_t[s, :, L:] = x[2 * bp + 1, ch].T
        mcore = dict(common)
        mcore["x_t"] = x_t
        mcore["p1t"] = np.ascontiguousarray(p1t)
        in_maps.append(mcore)
    return in_maps


def assemble_out(results):
    """results: list (per core) of dicts with 'out' [NSP, D, FP]."""
    y = np.empty((B, C, L, D), np.float32)
    for core in range(NCORES):
        o = results[core]["out"]
        for ci in range(CH_PER_CORE):
            ch = core * CH_PER_CORE + ci
            for bp in range(B // 2):
                s = ci * (B // 2) + bp
                y[2 * bp, ch] = o[s, :, :L].T
                y[2 * bp + 1, ch] = o[s, :, L:].T
    return y


_NC = None
LAST_RESULT = None


def kernel(x, w_qkv, pos_emb, w1, b1, w2, b2, w_out, b_out):
    global _NC, LAST_RESULT
    from concourse.bass_utils import run_bass_kernel_spmd

    if _NC is None:
        _NC = build_nc()
    in_maps = make_in_maps(x, w_qkv, pos_emb, w1, b1, w2, b2, w_out, b_out)
    res = run_bass_kernel_spmd(_NC, in_maps, core_ids=list(range(NCORES)))
    LAST_RESULT = res
    return assemble_out(res.results)


# revision 7
# speedup vs baseline: 1.6102x; 1.0908x over previous
"""Trainium2 Bass kernel: distance-decay double-softmax attention.

Reference computation per (b, c) pair (L=256, D=512):
    qkv  = x @ w_qkv;  q,k,v = split(qkv)
    attn = softmax(q @ k.T * D_h^-0.5)
    h    = relu((attn + pos) @ w1 + b1);  w = h @ w2 + b2
    attn2= softmax(attn * exp(-dist / (2 w^2 + 1e-6)))
    out  = (attn2 @ v) @ w_out + b_out

Host-side algebraic folds (exact):
    dots = q k^T * s = x (s Wq Wk^T) x^T         -> M = s*Wq@Wk.T
    y    = attn2 @ (v w_out) + b_out             -> Wv' = Wv@w_out
    (attn+pos) @ w1 + b1 = attn@w1 + (pos@w1+b1) -> P1[c] = pos[c]@w1+b1

Sharding: pure data parallel over the 128 (b,c) pairs -> 16 pairs/core,
packed as 8 "superpairs" (2 batch items of one channel share the free
dim, giving N=512 matmuls).  x arrives host-pretransposed; the output
leaves as y^T and is untransposed on the host.  attn / attn2 are
transposed on the PE (via identity).  All matmuls run as float32r
(full-rate fp32 storage) with fp32 PSUM accumulation.

Emission is software-pipelined across superpairs (stage A of superpair
sp is emitted before stage B of superpair sp-1) so the TensorEngine
never drains during the softmax/MLP chain and the HAM clock stays warm.
"""

import sys
import numpy as np

sys.path.insert(0, "/opt/trn_rl_repo")

import concourse.bass as bass  # noqa: E402,F401
import concourse.mybir as mybir  # noqa: E402
from concourse import bacc  # noqa: E402
from concourse.tile import TileContext  # noqa: E402

F32 = mybir.dt.float32
F32R = mybir.dt.float32r
AF = mybir.ActivationFunctionType
ALU = mybir.AluOpType

B, C, L, D = 8, 16, 256, 512
NCORES = 8
CH_PER_CORE = C // NCORES          # 2
NSP = (B // 2) * CH_PER_CORE       # 8 superpairs per core
P = 128
FP = 2 * L                         # 512: two pairs packed along free dim
DT = D // P                        # 4
LT = L // P                        # 2
SCALE = float(64 ** -0.5)          # DIM_HEAD ** -0.5


class _Ctx:
    pass


def _emit_stage_a(g, sp):
    """x load, t^T = (x M)^T, v' = x Wv', dots = t x^T, E=exp(dots)+rowsum."""
    nc, pp, sp_pool = g.nc, g.pp, g.apool
    MM = nc.tensor.matmul
    st = g.state[sp] = _Ctx()

    # x^T tiles [128(d), 512(l packed)]
    xt = []
    for dt in range(DT):
        t = sp_pool.tile([P, FP], F32R, tag=f"xt{dt}", name=f"xt{sp}_{dt}")
        nc.sync.dma_start(out=t[:, :], in_=g.h["x_t"][sp, dt * P:(dt + 1) * P, :])
        xt.append(t)
    st.xt = xt

    # t^T[e, l] = sum_d M[d, e] x^T[d, l]
    tT = []
    for et in range(DT):
        ps = pp.tile([P, FP], F32, tag="ps", name=f"ps_t{sp}_{et}")
        for dt in range(DT):
            MM(ps[:, :], g.m_sb[dt][:, et * P:(et + 1) * P], xt[dt][:, :],
               start=(dt == 0), stop=(dt == DT - 1))
        t = sp_pool.tile([P, FP], F32R, tag=f"tT{et}", name=f"tT{sp}_{et}")
        nc.vector.tensor_copy(t[:, :], ps[:, :])
        tT.append(t)
    st.tT = tT

    # v'[l, e] = sum_d x^T[d, l] Wv'[d, e]   (natural layout, per pair)
    v_sb = [[None] * LT for _ in range(2)]
    for pi in range(2):
        for lt in range(LT):
            ps = pp.tile([P, D], F32, tag="ps", name=f"ps_v{sp}_{pi}{lt}")
            for dt in range(DT):
                MM(ps[:, :],
                   xt[dt][:, pi * L + lt * P: pi * L + (lt + 1) * P],
                   g.wv_sb[dt][:, :],
                   start=(dt == 0), stop=(dt == DT - 1))
            t = sp_pool.tile([P, D], F32R, tag=f"v{pi}{lt}", name=f"v{sp}_{pi}{lt}")
            nc.vector.tensor_copy(t[:, :], ps[:, :])
            v_sb[pi][lt] = t
    st.v = v_sb

    # dots[i, m] = sum_e t^T[e, i] x^T[e, m]   (scale folded into M)
    dps = []
    for it in range(LT):
        ps = pp.tile([P, FP], F32, tag="ps", name=f"ps_d{sp}_{it}")
        for pi in range(2):
            o = ps[:, pi * L:(pi + 1) * L]
            for et in range(DT):
                MM(o,
                   tT[et][:, pi * L + it * P: pi * L + (it + 1) * P],
                   xt[et][:, pi * L:(pi + 1) * L],
                   start=(et == 0), stop=(et == DT - 1))
        dps.append(ps)

    # E = exp(dots), s1 = rowsum(E)
    s14 = sp_pool.tile([P, 4], F32, tag="s14", name=f"s14_{sp}")
    E = []
    for it in range(LT):
        e_t = sp_pool.tile([P, FP], F32, tag=f"E{it}", name=f"E{sp}_{it}")
        for pi in range(2):
            c = it * 2 + pi
            sl = slice(pi * L, (pi + 1) * L)
            nc.scalar.activation(e_t[:, sl], dps[it][:, sl], AF.Exp,
                                 accum_out=s14[:, c:c + 1])
        E.append(e_t)
    st.E = E
    r14 = sp_pool.tile([P, 4], F32, tag="r14", name=f"r14_{sp}")
    nc.vector.reciprocal(r14[:, :], s14[:, :])
    st.r14 = r14

    # attn = E * r1  (used by both the MLP transpose and the second softmax)
    attn = []
    for it in range(LT):
        t = sp_pool.tile([P, FP], F32, tag=f"at{it}", name=f"attn{sp}_{it}")
        for pi in range(2):
            c = it * 2 + pi
            sl = slice(pi * L, (pi + 1) * L)
            nc.vector.tensor_scalar_mul(t[:, sl], E[it][:, sl], r14[:, c:c + 1])
        attn.append(t)
    st.attn = attn


def _emit_stage_b1(g, sp):
    """transpose attn, MLP, dist-decay, softmax2 -> attn2 (in wg tiles)."""
    nc, pp, sp_pool = g.nc, g.pp, g.sp_pool
    MM = nc.tensor.matmul
    st = g.state[sp]
    ci = sp // (NSP // CH_PER_CORE)
    attn = st.attn

    # attn^T  [m(part), i(packed free)]
    aT = []
    for mt in range(LT):
        ps = pp.tile([P, FP], F32, tag="ps", name=f"ps_tA{sp}_{mt}")
        for pi in range(2):
            for it in range(LT):
                nc.tensor.transpose(
                    ps[:, pi * L + it * P: pi * L + (it + 1) * P],
                    attn[it][:, pi * L + mt * P: pi * L + (mt + 1) * P],
                    g.id_sb[:, :])
        t = sp_pool.tile([P, FP], F32R, tag=f"trT{mt}", name=f"aT{sp}_{mt}")
        nc.vector.tensor_copy(t[:, :], ps[:, :])
        aT.append(t)

    # h^T = relu(w1^T attn^T + P1^T)   [j(part), i(packed)]
    hT = []
    for jt in range(LT):
        ps = pp.tile([P, FP], F32, tag="ps", name=f"ps_h{sp}_{jt}")
        for mt in range(LT):
            MM(ps[:, :], g.w1_sb[mt][:, jt * P:(jt + 1) * P], aT[mt][:, :],
               start=(mt == 0), stop=False)
        MM(ps[:, :], g.idr_sb[:, :], g.p1_sb[ci][jt][:, :],
           start=False, stop=True)
        t = sp_pool.tile([P, FP], F32R, tag=f"hT{jt}", name=f"hT{sp}_{jt}")
        nc.scalar.activation(t[:, :], ps[:, :], AF.Relu)
        hT.append(t)

    # w[i] = h[i, :] @ w2 ; negt = -1/(2(w+b2)^2 + 1e-6)
    wps = pp.tile([P, 8], F32, tag="ps", name=f"ps_w{sp}")
    for pi in range(2):
        for it in range(LT):
            c = it * 2 + pi
            for jt in range(LT):
                MM(wps[:, 2 * c:2 * c + 2],
                   hT[jt][:, pi * L + it * P: pi * L + (it + 1) * P],
                   g.w2_sb[jt][:, :],
                   start=(jt == 0), stop=(jt == LT - 1))
    w4 = sp_pool.tile([P, 8], F32, tag="w4", name=f"w4_{sp}")
    nc.scalar.activation(w4[:, :], wps[:, :], AF.Square, bias=g.b2_sb[:, 0:1])
    nc.vector.tensor_scalar(w4[:, :], w4[:, :], -2.0, -1e-6, ALU.mult, ALU.add)
    negt = sp_pool.tile([P, 8], F32, tag="negt", name=f"negt_{sp}")
    nc.vector.reciprocal(negt[:, :], w4[:, :])

    # wg = exp(dist * negt); p2 = attn*wg; E2 = exp(p2) (+s2); attn2 = E2*r2
    s24 = sp_pool.tile([P, 4], F32, tag="s24", name=f"s24_{sp}")
    wg = []
    for it in range(LT):
        t = sp_pool.tile([P, FP], F32, tag=f"wg{it}", name=f"wg{sp}_{it}")
        for pi in range(2):
            c = it * 2 + pi
            sl = slice(pi * L, (pi + 1) * L)
            nc.scalar.activation(t[:, sl], g.dist_sb[it][:, sl], AF.Exp,
                                 scale=negt[:, 2 * c:2 * c + 1])
        nc.vector.tensor_mul(t[:, :], st.attn[it][:, :], t[:, :])
        for pi in range(2):
            c = it * 2 + pi
            sl = slice(pi * L, (pi + 1) * L)
            nc.scalar.activation(t[:, sl], t[:, sl], AF.Exp,
                                 accum_out=s24[:, c:c + 1])
        wg.append(t)
    r24 = sp_pool.tile([P, 4], F32, tag="r24", name=f"r24_{sp}")
    nc.vector.reciprocal(r24[:, :], s24[:, :])
    for it in range(LT):
        for pi in range(2):
            c = it * 2 + pi
            sl = slice(pi * L, (pi + 1) * L)
            nc.vector.tensor_scalar_mul(wg[it][:, sl], wg[it][:, sl],
                                        r24[:, c:c + 1])

    st.wg = wg


def _emit_stage_b2(g, sp):
    """transpose attn2, y^T matmuls, bias, DMA out."""
    nc, pp, sp_pool = g.nc, g.pp, g.sp_pool
    MM = nc.tensor.matmul
    st = g.state[sp]
    wg = st.wg

    # attn2^T [m(part), i(packed)]
    a2T = []
    for mt in range(LT):
        ps = pp.tile([P, FP], F32, tag="ps", name=f"ps_tB{sp}_{mt}")
        for pi in range(2):
            for it in range(LT):
                nc.tensor.transpose(
                    ps[:, pi * L + it * P: pi * L + (it + 1) * P],
                    wg[it][:, pi * L + mt * P: pi * L + (mt + 1) * P],
                    g.id_sb[:, :])
        t = sp_pool.tile([P, FP], F32R, tag=f"trT{mt}", name=f"a2T{sp}_{mt}")
        nc.vector.tensor_copy(t[:, :], ps[:, :])
        a2T.append(t)

    # y^T[d', i] = sum_m v'[m, d'] attn2^T[m, i] + b_out[d']
    for ot in range(DT):
        ps = pp.tile([P, FP], F32, tag="ps", name=f"ps_y{sp}_{ot}")
        for pi in range(2):
            o = ps[:, pi * L:(pi + 1) * L]
            for mt in range(LT):
                MM(o,
                   st.v[pi][mt][:, ot * P:(ot + 1) * P],
                   a2T[mt][:, pi * L:(pi + 1) * L],
                   start=(mt == 0), stop=(mt == LT - 1))
        yt = g.ypool.tile([P, FP], F32, tag=f"yT{ot}", name=f"yT{sp}_{ot}")
        nc.scalar.activation(yt[:, :], ps[:, :], AF.Identity,
                             bias=g.bo_sb[ot][:, 0:1])
        nc.sync.dma_start(out=g.h["out"][sp, ot * P:(ot + 1) * P, :],
                          in_=yt[:, :])


def _emit(nc, tc, h):
    import contextlib
    g = _Ctx()
    g.nc, g.h = nc, h
    g.state = {}

    with contextlib.ExitStack() as ex:
        cpool = ex.enter_context(tc.tile_pool(name="consts", bufs=1))
        g.apool = ex.enter_context(tc.tile_pool(name="astream", bufs=3))
        g.sp_pool = ex.enter_context(tc.tile_pool(name="stream", bufs=2))
        g.ypool = ex.enter_context(tc.tile_pool(name="yout", bufs=1))
        g.pp = ex.enter_context(tc.tile_pool(name="ps", bufs=8, space="PSUM"))

        # ---- constants ----
        def cload(name, shape, dt_, src):
            t = cpool.tile(shape, dt_, tag=name, name=name)
            nc.sync.dma_start(out=t[:shape[0], :], in_=src)
            return t

        # Stage-A consts first so the PE can start as soon as m/xt land;
        # everything stage-B needs streams in behind the first A stages.
        g.m_sb = [cload(f"m{dt}", [P, D], F32R, h["m"][dt * P:(dt + 1) * P, :])
                  for dt in range(DT)]
        g.wv_sb = [cload(f"wv{dt}", [P, D], F32R, h["wv"][dt * P:(dt + 1) * P, :])
                   for dt in range(DT)]

        def late_consts():
            g.w1_sb = [cload(f"w1_{mt}", [P, L], F32R,
                             h["w1"][mt * P:(mt + 1) * P, :])
                       for mt in range(LT)]
            g.w2_sb = [cload(f"w2_{jt}", [P, 2], F32R,
                             h["w2d"][jt * P:(jt + 1) * P, :])
                       for jt in range(LT)]
            g.p1_sb = [[cload(f"p1_{ci}_{jt}", [P, FP], F32R,
                              h["p1t"][ci, jt * P:(jt + 1) * P, :])
                        for jt in range(LT)] for ci in range(CH_PER_CORE)]
            g.bo_sb = [cload(f"bout{ot}", [P, 1], F32,
                             h["bout"][ot * P:(ot + 1) * P, :])
                       for ot in range(DT)]
            g.b2_sb = cload("b2r", [P, 1], F32, h["b2r"][:, :])
            g.id_sb = cload("ident", [P, P], F32, h["ident"][:, :])
            g.idr_sb = cload("identr", [P, P], F32R, h["identr"][:, :])
            g.dist_sb = [cload(f"dist{it}", [P, FP], F32,
                               h["dist"][it * P:(it + 1) * P, :])
                         for it in range(LT)]

        # ---- software-pipelined superpair loop ----
        # PE stream per period: [B1(sp)] [A(sp+2): 48 independent MMs]
        # [B2(sp)] -- the A block covers the softmax/MLP chain latency so
        # B2's transposes never stall the PE.  The tail (no A left) runs
        # B1(6), B1(7), B2(6), B2(7) so B1(7)'s matmuls cover B2(6)'s chain.
        _emit_stage_a(g, 0)
        late_consts()
        _emit_stage_a(g, 1)
        for sp in range(NSP - 2):
            _emit_stage_b1(g, sp)
            _emit_stage_a(g, sp + 2)
            _emit_stage_b2(g, sp)
        _emit_stage_b1(g, NSP - 2)
        _emit_stage_b1(g, NSP - 1)
        _emit_stage_b2(g, NSP - 2)
        _emit_stage_b2(g, NSP - 1)


def build_nc():
    nc = bacc.Bacc("TRN2", target_bir_lowering=False, debug=False,
                   enable_asserts=False)
    h = {}
    h["x_t"] = nc.declare_dram_parameter("x_t", [NSP, D, FP], F32R, False)
    h["m"] = nc.declare_dram_parameter("m", [D, D], F32R, False)
    h["wv"] = nc.declare_dram_parameter("wv", [D, D], F32R, False)
    h["w1"] = nc.declare_dram_parameter("w1", [L, L], F32R, False)
    h["w2d"] = nc.declare_dram_parameter("w2d", [L, 2], F32R, False)
    h["p1t"] = nc.declare_dram_parameter("p1t", [CH_PER_CORE, L, FP], F32R, False)
    h["dist"] = nc.declare_dram_parameter("dist", [L, FP], F32, False)
    h["bout"] = nc.declare_dram_parameter("bout", [D, 1], F32, False)
    h["b2r"] = nc.declare_dram_parameter("b2r", [P, 1], F32, False)
    h["ident"] = nc.declare_dram_parameter("ident", [P, P], F32, False)
    h["identr"] = nc.declare_dram_parameter("identr", [P, P], F32R, False)
    h["out"] = nc.declare_dram_parameter("out", [NSP, D, FP], F32, True)

    with TileContext(nc) as tc:
        _emit(nc, tc, h)
    nc.compile()
    return nc


def make_in_maps(x, w_qkv, pos_emb, w1, b1, w2, b2, w_out, b_out):
    f = lambda a: np.ascontiguousarray(np.asarray(a), dtype=np.float32)
    x, w_qkv, pos_emb = f(x), f(w_qkv), f(pos_emb)
    w1, b1, w2, b2, w_out, b_out = f(w1), f(b1), f(w2), f(b2), f(w_out), f(b_out)

    wq, wk, wv = w_qkv[:, :D], w_qkv[:, D:2 * D], w_qkv[:, 2 * D:]
    m = np.ascontiguousarray((SCALE * (wq.astype(np.float64)
                                       @ wk.astype(np.float64).T))
                             .astype(np.float32))
    wvp = np.ascontiguousarray((wv.astype(np.float64)
                                @ w_out.astype(np.float64)).astype(np.float32))
    # P1[c] = pos[c] @ w1 + b1, transposed [L(j), L(i)] per channel
    p1 = pos_emb[0].astype(np.float64) @ w1.astype(np.float64) + b1
    p1t_single = np.ascontiguousarray(p1.transpose(0, 2, 1).astype(np.float32))
    idx = np.arange(L, dtype=np.float32)
    dist = (idx[None, :] - idx[:, None]) ** 2
    distp = np.ascontiguousarray(np.concatenate([dist, dist], axis=1))
    common = {
        "m": m,
        "wv": wvp,
        "w1": w1,
        "w2d": np.ascontiguousarray(np.concatenate([w2, w2], axis=1)),
        "dist": distp,
        "bout": np.ascontiguousarray(b_out.reshape(D, 1)),
        "b2r": np.full((P, 1), b2.reshape(-1)[0], np.float32),
        "ident": np.eye(P, dtype=np.float32),
        "identr": np.eye(P, dtype=np.float32),
    }
    in_maps = []
    for core in range(NCORES):
        x_t = np.empty((NSP, D, FP), np.float32)
        p1t = np.empty((CH_PER_CORE, L, FP), np.float32)
        for ci in range(CH_PER_CORE):
            ch = core * CH_PER_CORE + ci
            p1t[ci, :, :L] = p1t_single[ch]
            p1t[ci, :, L:] = p1t_single[ch]
            for bp in range(B // 2):
                s = ci * (B // 2) + bp
                x_t[s, :, :L] = x[2 * bp, ch].T
                x_t[s, :, L:] = x[2 * bp + 1, ch].T
        mcore = dict(common)
        mcore["x_t"] = x_t
        mcore["p1t"] = np.ascontiguousarray(p1t)
        in_maps.append(mcore)
    return in_maps


def assemble_out(results):
    """results: list (per core) of dicts with 'out' [NSP, D, FP]."""
    y = np.empty((B, C, L, D), np.float32)
    for core in range(NCORES):
        o = results[core]["out"]
        for ci in range(CH_PER_CORE):
            ch = core * CH_PER_CORE + ci
            for bp in range(B // 2):
                s = ci * (B // 2) + bp
                y[2 * bp, ch] = o[s, :, :L].T
                y[2 * bp + 1, ch] = o[s, :, L:].T
    return y


_NC = None
LAST_RESULT = None


def kernel(x, w_qkv, pos_emb, w1, b1, w2, b2, w_out, b_out):
    global _NC, LAST_RESULT
    from concourse.bass_utils import run_bass_kernel_spmd

    if _NC is None:
        _NC = build_nc()
    in_maps = make_in_maps(x, w_qkv, pos_emb, w1, b1, w2, b2, w_out, b_out)
    res = run_bass_kernel_spmd(_NC, in_maps, core_ids=list(range(NCORES)))
    LAST_RESULT = res
    return assemble_out(res.results)


# revision 8
# speedup vs baseline: 1.6287x; 1.0115x over previous
"""Trainium2 Bass kernel: distance-decay double-softmax attention.

Reference computation per (b, c) pair (L=256, D=512):
    qkv  = x @ w_qkv;  q,k,v = split(qkv)
    attn = softmax(q @ k.T * D_h^-0.5)
    h    = relu((attn + pos) @ w1 + b1);  w = h @ w2 + b2
    attn2= softmax(attn * exp(-dist / (2 w^2 + 1e-6)))
    out  = (attn2 @ v) @ w_out + b_out

Host-side algebraic folds (exact):
    dots = q k^T * s = x (s Wq Wk^T) x^T         -> M = s*Wq@Wk.T
    y    = attn2 @ (v w_out) + b_out             -> Wv' = Wv@w_out
    (attn+pos) @ w1 + b1 = attn@w1 + (pos@w1+b1) -> P1[c] = pos[c]@w1+b1

Sharding: pure data parallel over the 128 (b,c) pairs -> 16 pairs/core,
packed as 8 "superpairs" (2 batch items of one channel share the free
dim, giving N=512 matmuls).  x arrives host-pretransposed; the output
leaves as y^T and is untransposed on the host.  attn / attn2 are
transposed on the PE (via identity).  All matmuls run as float32r
(full-rate fp32 storage) with fp32 PSUM accumulation.

Emission is software-pipelined across superpairs (stage A of superpair
sp is emitted before stage B of superpair sp-1) so the TensorEngine
never drains during the softmax/MLP chain and the HAM clock stays warm.
"""

import sys
import numpy as np

sys.path.insert(0, "/opt/trn_rl_repo")

import concourse.bass as bass  # noqa: E402,F401
import concourse.mybir as mybir  # noqa: E402
from concourse import bacc  # noqa: E402
from concourse.tile import TileContext  # noqa: E402

F32 = mybir.dt.float32
F32R = mybir.dt.float32r
AF = mybir.ActivationFunctionType
ALU = mybir.AluOpType

B, C, L, D = 8, 16, 256, 512
NCORES = 8
CH_PER_CORE = C // NCORES          # 2
NSP = (B // 2) * CH_PER_CORE       # 8 superpairs per core
P = 128
FP = 2 * L                         # 512: two pairs packed along free dim
DT = D // P                        # 4
LT = L // P                        # 2
SCALE = float(64 ** -0.5)          # DIM_HEAD ** -0.5


class _Ctx:
    pass


def _emit_stage_a(g, sp):
    """x load, t^T = (x M)^T, v' = x Wv', dots = t x^T, E=exp(dots)+rowsum."""
    nc, pp, sp_pool = g.nc, g.pp, g.apool
    MM = nc.tensor.matmul
    st = g.state[sp] = _Ctx()

    # x^T tiles [128(d), 512(l packed)]
    xt = []
    for dt in range(DT):
        t = sp_pool.tile([P, FP], F32R, tag=f"xt{dt}", name=f"xt{sp}_{dt}")
        nc.sync.dma_start(out=t[:, :], in_=g.h["x_t"][sp, dt * P:(dt + 1) * P, :])
        xt.append(t)
    st.xt = xt

    # t^T[e, l] = sum_d M[d, e] x^T[d, l]
    tT = []
    for et in range(DT):
        ps = pp.tile([P, FP], F32, tag="ps", name=f"ps_t{sp}_{et}")
        for dt in range(DT):
            MM(ps[:, :], g.m_sb[dt][:, et * P:(et + 1) * P], xt[dt][:, :],
               start=(dt == 0), stop=(dt == DT - 1))
        t = sp_pool.tile([P, FP], F32R, tag=f"tT{et}", name=f"tT{sp}_{et}")
        nc.vector.tensor_copy(t[:, :], ps[:, :])
        tT.append(t)
    st.tT = tT

    # v'[l, e] = sum_d x^T[d, l] Wv'[d, e]   (natural layout, per pair)
    v_sb = [[None] * LT for _ in range(2)]
    for pi in range(2):
        for lt in range(LT):
            ps = pp.tile([P, D], F32, tag="ps", name=f"ps_v{sp}_{pi}{lt}")
            for dt in range(DT):
                MM(ps[:, :],
                   xt[dt][:, pi * L + lt * P: pi * L + (lt + 1) * P],
                   g.wv_sb[dt][:, :],
                   start=(dt == 0), stop=(dt == DT - 1))
            t = sp_pool.tile([P, D], F32R, tag=f"v{pi}{lt}", name=f"v{sp}_{pi}{lt}")
            nc.vector.tensor_copy(t[:, :], ps[:, :])
            v_sb[pi][lt] = t
    st.v = v_sb

    # dots[i, m] = sum_e t^T[e, i] x^T[e, m]   (scale folded into M)
    dps = []
    for it in range(LT):
        ps = pp.tile([P, FP], F32, tag="ps", name=f"ps_d{sp}_{it}")
        for pi in range(2):
            o = ps[:, pi * L:(pi + 1) * L]
            for et in range(DT):
                MM(o,
                   tT[et][:, pi * L + it * P: pi * L + (it + 1) * P],
                   xt[et][:, pi * L:(pi + 1) * L],
                   start=(et == 0), stop=(et == DT - 1))
        dps.append(ps)

    # E = exp(dots), s1 = rowsum(E)
    s14 = sp_pool.tile([P, 4], F32, tag="s14", name=f"s14_{sp}")
    E = []
    for it in range(LT):
        e_t = sp_pool.tile([P, FP], F32, tag=f"E{it}", name=f"E{sp}_{it}")
        for pi in range(2):
            c = it * 2 + pi
            sl = slice(pi * L, (pi + 1) * L)
            nc.scalar.activation(e_t[:, sl], dps[it][:, sl], AF.Exp,
                                 accum_out=s14[:, c:c + 1])
        E.append(e_t)
    st.E = E
    r14 = sp_pool.tile([P, 4], F32, tag="r14", name=f"r14_{sp}")
    nc.vector.reciprocal(r14[:, :], s14[:, :])
    st.r14 = r14

    # attn = E * r1  (used by both the MLP transpose and the second softmax)
    attn = []
    for it in range(LT):
        t = sp_pool.tile([P, FP], F32, tag=f"at{it}", name=f"attn{sp}_{it}")
        for pi in range(2):
            c = it * 2 + pi
            sl = slice(pi * L, (pi + 1) * L)
            nc.vector.tensor_scalar_mul(t[:, sl], E[it][:, sl], r14[:, c:c + 1])
        attn.append(t)
    st.attn = attn


def _emit_stage_b1(g, sp):
    """transpose attn, MLP, dist-decay, softmax2 -> attn2 (in wg tiles)."""
    nc, pp, sp_pool = g.nc, g.pp, g.sp_pool
    MM = nc.tensor.matmul
    st = g.state[sp]
    ci = sp // (NSP // CH_PER_CORE)
    attn = st.attn

    # attn^T  [m(part), i(packed free)]
    aT = []
    for mt in range(LT):
        ps = pp.tile([P, FP], F32, tag="ps", name=f"ps_tA{sp}_{mt}")
        for pi in range(2):
            for it in range(LT):
                nc.tensor.transpose(
                    ps[:, pi * L + it * P: pi * L + (it + 1) * P],
                    attn[it][:, pi * L + mt * P: pi * L + (mt + 1) * P],
                    g.id_sb[:, :])
        t = sp_pool.tile([P, FP], F32R, tag=f"trT{mt}", name=f"aT{sp}_{mt}")
        nc.vector.tensor_copy(t[:, :], ps[:, :])
        aT.append(t)

    # h^T = relu(w1^T attn^T + P1^T)   [j(part), i(packed)]
    hT = []
    for jt in range(LT):
        ps = pp.tile([P, FP], F32, tag="ps", name=f"ps_h{sp}_{jt}")
        for mt in range(LT):
            MM(ps[:, :], g.w1_sb[mt][:, jt * P:(jt + 1) * P], aT[mt][:, :],
               start=(mt == 0), stop=False)
        MM(ps[:, :], g.idr_sb[:, :], g.p1_sb[ci][jt][:, :],
           start=False, stop=True)
        t = sp_pool.tile([P, FP], F32R, tag=f"hT{jt}", name=f"hT{sp}_{jt}")
        nc.scalar.activation(t[:, :], ps[:, :], AF.Relu)
        hT.append(t)

    # w[i] = h[i, :] @ w2 ; negt = -1/(2(w+b2)^2 + 1e-6)
    wps = pp.tile([P, 8], F32, tag="ps", name=f"ps_w{sp}")
    for pi in range(2):
        for it in range(LT):
            c = it * 2 + pi
            for jt in range(LT):
                MM(wps[:, 2 * c:2 * c + 2],
                   hT[jt][:, pi * L + it * P: pi * L + (it + 1) * P],
                   g.w2_sb[jt][:, :],
                   start=(jt == 0), stop=(jt == LT - 1))
    w4 = sp_pool.tile([P, 8], F32, tag="w4", name=f"w4_{sp}")
    nc.scalar.activation(w4[:, :], wps[:, :], AF.Square, bias=g.b2_sb[:, 0:1])
    nc.vector.tensor_scalar(w4[:, :], w4[:, :], -2.0, -1e-6, ALU.mult, ALU.add)
    negt = sp_pool.tile([P, 8], F32, tag="negt", name=f"negt_{sp}")
    nc.vector.reciprocal(negt[:, :], w4[:, :])

    # wg = exp(dist * negt); p2 = attn*wg; E2 = exp(p2) (+s2); attn2 = E2*r2
    s24 = sp_pool.tile([P, 4], F32, tag="s24", name=f"s24_{sp}")
    wg = []
    for it in range(LT):
        t = sp_pool.tile([P, FP], F32, tag=f"wg{it}", name=f"wg{sp}_{it}")
        for pi in range(2):
            c = it * 2 + pi
            sl = slice(pi * L, (pi + 1) * L)
            nc.scalar.activation(t[:, sl], g.dist_sb[it][:, sl], AF.Exp,
                                 scale=negt[:, 2 * c:2 * c + 1])
        nc.vector.tensor_mul(t[:, :], st.attn[it][:, :], t[:, :])
        for pi in range(2):
            c = it * 2 + pi
            sl = slice(pi * L, (pi + 1) * L)
            nc.scalar.activation(t[:, sl], t[:, sl], AF.Exp,
                                 accum_out=s24[:, c:c + 1])
        wg.append(t)
    r24 = sp_pool.tile([P, 4], F32, tag="r24", name=f"r24_{sp}")
    nc.vector.reciprocal(r24[:, :], s24[:, :])
    for it in range(LT):
        for pi in range(2):
            c = it * 2 + pi
            sl = slice(pi * L, (pi + 1) * L)
            nc.vector.tensor_scalar_mul(wg[it][:, sl], wg[it][:, sl],
                                        r24[:, c:c + 1])

    st.wg = wg


def _emit_stage_b2(g, sp):
    """transpose attn2, y^T matmuls, bias, DMA out."""
    nc, pp, sp_pool = g.nc, g.pp, g.sp_pool
    MM = nc.tensor.matmul
    st = g.state[sp]
    wg = st.wg

    # attn2^T [m(part), i(packed)]
    a2T = []
    for mt in range(LT):
        ps = pp.tile([P, FP], F32, tag="ps", name=f"ps_tB{sp}_{mt}")
        for pi in range(2):
            for it in range(LT):
                nc.tensor.transpose(
                    ps[:, pi * L + it * P: pi * L + (it + 1) * P],
                    wg[it][:, pi * L + mt * P: pi * L + (mt + 1) * P],
                    g.id_sb[:, :])
        t = sp_pool.tile([P, FP], F32R, tag=f"trT{mt}", name=f"a2T{sp}_{mt}")
        nc.vector.tensor_copy(t[:, :], ps[:, :])
        a2T.append(t)

    # y^T[d', i] = sum_m v'[m, d'] attn2^T[m, i] + b_out[d']
    for ot in range(DT):
        ps = pp.tile([P, FP], F32, tag="ps", name=f"ps_y{sp}_{ot}")
        for pi in range(2):
            o = ps[:, pi * L:(pi + 1) * L]
            for mt in range(LT):
                MM(o,
                   st.v[pi][mt][:, ot * P:(ot + 1) * P],
                   a2T[mt][:, pi * L:(pi + 1) * L],
                   start=(mt == 0), stop=(mt == LT - 1))
        yt = g.ypool.tile([P, FP], F32, tag=f"yT{ot}", name=f"yT{sp}_{ot}")
        nc.scalar.activation(yt[:, :], ps[:, :], AF.Identity,
                             bias=g.bo_sb[ot][:, 0:1])
        nc.sync.dma_start(out=g.h["out"][sp, ot * P:(ot + 1) * P, :],
                          in_=yt[:, :])


def _emit(nc, tc, h):
    import contextlib
    g = _Ctx()
    g.nc, g.h = nc, h
    g.state = {}

    with contextlib.ExitStack() as ex:
        cpool = ex.enter_context(tc.tile_pool(name="consts", bufs=1))
        g.apool = ex.enter_context(tc.tile_pool(name="astream", bufs=3))
        g.sp_pool = ex.enter_context(tc.tile_pool(name="stream", bufs=2))
        g.ypool = ex.enter_context(tc.tile_pool(name="yout", bufs=1))
        g.pp = ex.enter_context(tc.tile_pool(name="ps", bufs=8, space="PSUM"))

        # ---- constants ----
        def cload(name, shape, dt_, src):
            t = cpool.tile(shape, dt_, tag=name, name=name)
            nc.sync.dma_start(out=t[:shape[0], :], in_=src)
            return t

        # Identity first, then ~20 dummy matmuls during the input-DMA head:
        # the HAM clock gate needs ~3.4us of sustained PE activity to lift
        # the 1.2GHz cold throttle, so warm it up while the PE would idle.
        g.idr_sb = cload("identr", [P, P], F32R, h["identr"][:, :])
        warm_ps = g.pp.tile([P, P], F32, tag="ps", name="warmup_ps")
        for wi in range(20):
            nc.tensor.matmul(warm_ps[:, :], g.idr_sb[:, :], g.idr_sb[:, :],
                             start=True, stop=True)

        # Stage-A consts first so the PE can start as soon as m/xt land;
        # everything stage-B needs streams in behind the first A stages.
        g.m_sb = [cload(f"m{dt}", [P, D], F32R, h["m"][dt * P:(dt + 1) * P, :])
                  for dt in range(DT)]
        g.wv_sb = [cload(f"wv{dt}", [P, D], F32R, h["wv"][dt * P:(dt + 1) * P, :])
                   for dt in range(DT)]

        def late_consts():
            g.w1_sb = [cload(f"w1_{mt}", [P, L], F32R,
                             h["w1"][mt * P:(mt + 1) * P, :])
                       for mt in range(LT)]
            g.w2_sb = [cload(f"w2_{jt}", [P, 2], F32R,
                             h["w2d"][jt * P:(jt + 1) * P, :])
                       for jt in range(LT)]
            g.p1_sb = [[cload(f"p1_{ci}_{jt}", [P, FP], F32R,
                              h["p1t"][ci, jt * P:(jt + 1) * P, :])
                        for jt in range(LT)] for ci in range(CH_PER_CORE)]
            g.bo_sb = [cload(f"bout{ot}", [P, 1], F32,
                             h["bout"][ot * P:(ot + 1) * P, :])
                       for ot in range(DT)]
            g.b2_sb = cload("b2r", [P, 1], F32, h["b2r"][:, :])
            g.id_sb = cload("ident", [P, P], F32, h["ident"][:, :])
            g.dist_sb = [cload(f"dist{it}", [P, FP], F32,
                               h["dist"][it * P:(it + 1) * P, :])
                         for it in range(LT)]

        # ---- software-pipelined superpair loop ----
        # PE stream per period: [B1(sp)] [A(sp+2): 48 independent MMs]
        # [B2(sp)] -- the A block covers the softmax/MLP chain latency so
        # B2's transposes never stall the PE.  The tail (no A left) runs
        # B1(6), B1(7), B2(6), B2(7) so B1(7)'s matmuls cover B2(6)'s chain.
        _emit_stage_a(g, 0)
        late_consts()
        _emit_stage_a(g, 1)
        for sp in range(NSP - 2):
            _emit_stage_b1(g, sp)
            _emit_stage_a(g, sp + 2)
            _emit_stage_b2(g, sp)
        _emit_stage_b1(g, NSP - 2)
        _emit_stage_b1(g, NSP - 1)
        _emit_stage_b2(g, NSP - 2)
        _emit_stage_b2(g, NSP - 1)


def build_nc():
    nc = bacc.Bacc("TRN2", target_bir_lowering=False, debug=False,
                   enable_asserts=False)
    h = {}
    h["x_t"] = nc.declare_dram_parameter("x_t", [NSP, D, FP], F32R, False)
    h["m"] = nc.declare_dram_parameter("m", [D, D], F32R, False)
    h["wv"] = nc.declare_dram_parameter("wv", [D, D], F32R, False)
    h["w1"] = nc.declare_dram_parameter("w1", [L, L], F32R, False)
    h["w2d"] = nc.declare_dram_parameter("w2d", [L, 2], F32R, False)
    h["p1t"] = nc.declare_dram_parameter("p1t", [CH_PER_CORE, L, FP], F32R, False)
    h["dist"] = nc.declare_dram_parameter("dist", [L, FP], F32, False)
    h["bout"] = nc.declare_dram_parameter("bout", [D, 1], F32, False)
    h["b2r"] = nc.declare_dram_parameter("b2r", [P, 1], F32, False)
    h["ident"] = nc.declare_dram_parameter("ident", [P, P], F32, False)
    h["identr"] = nc.declare_dram_parameter("identr", [P, P], F32R, False)
    h["out"] = nc.declare_dram_parameter("out", [NSP, D, FP], F32, True)

    with TileContext(nc) as tc:
        _emit(nc, tc, h)
    nc.compile()
    return nc


def make_in_maps(x, w_qkv, pos_emb, w1, b1, w2, b2, w_out, b_out):
    f = lambda a: np.ascontiguousarray(np.asarray(a), dtype=np.float32)
    x, w_qkv, pos_emb = f(x), f(w_qkv), f(pos_emb)
    w1, b1, w2, b2, w_out, b_out = f(w1), f(b1), f(w2), f(b2), f(w_out), f(b_out)

    wq, wk, wv = w_qkv[:, :D], w_qkv[:, D:2 * D], w_qkv[:, 2 * D:]
    m = np.ascontiguousarray((SCALE * (wq.astype(np.float64)
                                       @ wk.astype(np.float64).T))
                             .astype(np.float32))
    wvp = np.ascontiguousarray((wv.astype(np.float64)
                                @ w_out.astype(np.float64)).astype(np.float32))
    # P1[c] = pos[c] @ w1 + b1, transposed [L(j), L(i)] per channel
    p1 = pos_emb[0].astype(np.float64) @ w1.astype(np.float64) + b1
    p1t_single = np.ascontiguousarray(p1.transpose(0, 2, 1).astype(np.float32))
    idx = np.arange(L, dtype=np.float32)
    dist = (idx[None, :] - idx[:, None]) ** 2
    distp = np.ascontiguousarray(np.concatenate([dist, dist], axis=1))
    common = {
        "m": m,
        "wv": wvp,
        "w1": w1,
        "w2d": np.ascontiguousarray(np.concatenate([w2, w2], axis=1)),
        "dist": distp,
        "bout": np.ascontiguousarray(b_out.reshape(D, 1)),
        "b2r": np.full((P, 1), b2.reshape(-1)[0], np.float32),
        "ident": np.eye(P, dtype=np.float32),
        "identr": np.eye(P, dtype=np.float32),
    }
    in_maps = []
    for core in range(NCORES):
        x_t = np.empty((NSP, D, FP), np.float32)
        p1t = np.empty((CH_PER_CORE, L, FP), np.float32)
        for ci in range(CH_PER_CORE):
            ch = core * CH_PER_CORE + ci
            p1t[ci, :, :L] = p1t_single[ch]
            p1t[ci, :, L:] = p1t_single[ch]
            for bp in range(B // 2):
                s = ci * (B // 2) + bp
                x_t[s, :, :L] = x[2 * bp, ch].T
                x_t[s, :, L:] = x[2 * bp + 1, ch].T
        mcore = dict(common)
        mcore["x_t"] = x_t
        mcore["p1t"] = np.ascontiguousarray(p1t)
        in_maps.append(mcore)
    return in_maps


def assemble_out(results):
    """results: list (per core) of dicts with 'out' [NSP, D, FP]."""
    y = np.empty((B, C, L, D), np.float32)
    for core in range(NCORES):
        o = results[core]["out"]
        for ci in range(CH_PER_CORE):
            ch = core * CH_PER_CORE + ci
            for bp in range(B // 2):
                s = ci * (B // 2) + bp
                y[2 * bp, ch] = o[s, :, :L].T
                y[2 * bp + 1, ch] = o[s, :, L:].T
    return y


_NC = None
LAST_RESULT = None


def kernel(x, w_qkv, pos_emb, w1, b1, w2, b2, w_out, b_out):
    global _NC, LAST_RESULT
    from concourse.bass_utils import run_bass_kernel_spmd

    if _NC is None:
        _NC = build_nc()
    in_maps = make_in_maps(x, w_qkv, pos_emb, w1, b1, w2, b2, w_out, b_out)
    res = run_bass_kernel_spmd(_NC, in_maps, core_ids=list(range(NCORES)))
    LAST_RESULT = res
    return assemble_out(res.results)
